# revision 27
# baseline (speedup 1.0000x reference)
"""Trainium2 Bass kernel for a GPT-2 style transformer block.

Problem: B=8, T=1024, C=768, H=12 heads, causal attention, GELU-tanh MLP.
Sharding: data-parallel over batch -- one batch element per NeuronCore,
weights replicated, no collectives.

Host-side prep (in kernel(), plain numpy):
  - LN1 gamma/beta folded into w_attn/b_attn; LN2 into w_fc1/b_fc1, so
    on-device LN is just (x - mu) * rsigma (one tensor_scalar op).
  - Weights repacked into SBUF-layout [128, ...] bf16 tensors so each
    weight tile is one large contiguous DMA.

Per-core dataflow (token tiles of 128, feature tiles of 128):
  P1  LN1 (bn_stats/bn_aggr fp32) token-major; DMA-transpose h -> hT
      (feature-major bf16); V = h @ Wv packed per head as [v | ones],
      interleaved per token tile so the PE starts early.
  P2  per head-pair: Q^T,K^T feature-major; scores computed transposed
      S^T[s,t] = K^T.T@Q^T with the two heads of a pair issued to PE
      row-groups 0/64 (K=64 matmuls run concurrently); exp(0.125*S) on
      ACT straight out of PSUM; causal diagonal mask on GpSimd;
      att^T @ [v|ones] gives y^T + softmax row-sums in the same
      matmuls; row-sum reciprocal via DVE reciprocal_approx_fast; PE
      ones-broadcast to all 64 rows; normalize into YT.
  P3  proj token-major, b_proj added via a K=1 ones-matmul, +residual,
      LN2 fused, DMA-transpose h2 -> h2T.
  P4  MLP in 3 strip-pairs (1024 hidden each): fc1+GELU hidden-major,
      fc2 accumulates 8 matmuls in PSUM (b_fc2 via K=1 ones-matmul on
      the first strip-pair), residual accumulated in SBUF fp32.

Matmul operands bf16 (fp32 PSUM); LN stats, residual stream, softmax
reciprocals fp32.
"""

import sys
from contextlib import ExitStack

if "/opt/trn_rl_repo" not in sys.path:
    sys.path.insert(0, "/opt/trn_rl_repo")

import ml_dtypes
import numpy as np

import concourse.bass as bass
import concourse.bacc as bacc
import concourse.mybir as mybir
import concourse.tile as tile
from concourse.bass_utils import run_bass_kernel_spmd
from concourse.masks import make_upper_triangular

P = 128
T = 1024
C = 768
H = 12
D = 64
F = 3072
TT = T // P      # 8 token tiles
KC = C // P      # 6 feature tiles
NP = H // 2      # 6 head pairs
NS = 3           # MLP strip-pairs (1024 hidden each)
LN_EPS = 1e-5
f32 = mybir.dt.float32
bf16 = mybir.dt.bfloat16
AF = mybir.ActivationFunctionType
ALU = mybir.AluOpType

N_CORES = 8

INPUT_NAMES = [
    "x", "ln1_g", "ln1_b", "w_attn", "b_attn", "w_proj", "b_proj",
    "ln2_g", "ln2_b", "w_fc1", "b_fc1", "w_fc2", "b_fc2",
]


def _layer_norm(nc, tmp, x_ap, out_h, eps_ap):
    """out_h = (x - mean(x)) * rsqrt(var(x)+eps) over the 768-wide free dim."""
    stats = tmp.tile([P, 3, 6], f32, tag="lnstats")
    xv = x_ap.rearrange("p (a b) -> p a b", b=256)
    for a in range(3):
        nc.vector.bn_stats(out=stats[:, a, :], in_=xv[:, a, :])
    mv = tmp.tile([P, 2], f32, tag="lnmv")
    nc.vector.bn_aggr(out=mv[:], in_=stats[:])
    rs = tmp.tile([P, 1], f32, tag="lnrs")
    nc.scalar.activation(out=rs[:], in_=mv[:, 1:2], func=AF.Sqrt,
                         bias=eps_ap, scale=1.0)
    rsr = tmp.tile([P, 1], f32, tag="lnrsr")
    nc.vector.reciprocal(out=rsr[:], in_=rs[:])
    nc.vector.tensor_scalar(out=out_h, in0=x_ap, scalar1=mv[:, 0:1],
                            scalar2=rsr[:], op0=ALU.subtract, op1=ALU.mult)


USE_GPSIMD_MASK = True
USE_BIAS_MM = True
USE_RECIP_APPROX = True


def build_nc():
    nc = bacc.Bacc("TRN2", target_bir_lowering=False, debug=False)

    x_d = nc.dram_tensor("x", [T, C], f32, kind="ExternalInput").ap()
    wqk_d = nc.dram_tensor("wqk", [P, NP * KC * 2 * P], bf16, kind="ExternalInput").ap()
    wv_d = nc.dram_tensor("wv", [P, KC * C], bf16, kind="ExternalInput").ap()
    bqk_d = nc.dram_tensor("bqk", [P, 2 * NP], f32, kind="ExternalInput").ap()
    bv_d = nc.dram_tensor("bv", [C], f32, kind="ExternalInput").ap()
    wp_d = nc.dram_tensor("wp", [P, KC * C], bf16, kind="ExternalInput").ap()
    bp_d = nc.dram_tensor("bp", [1, C], bf16, kind="ExternalInput").ap()
    w1_d = nc.dram_tensor("w1", [P, NS * KC * 1024], bf16, kind="ExternalInput").ap()
    b1_d = nc.dram_tensor("b1", [P, F // P], f32, kind="ExternalInput").ap()
    w2_d = nc.dram_tensor("w2", [P, NS * 8 * C], bf16, kind="ExternalInput").ap()
    b2_d = nc.dram_tensor("b2", [1, C], bf16, kind="ExternalInput").ap()
    warm_d = nc.dram_tensor("warm", [1, 1], f32, kind="ExternalOutput").ap()
    out_d = nc.dram_tensor("out", [T, C], f32, kind="ExternalOutput").ap()

    with tile.TileContext(nc) as tc, ExitStack() as es:
        if True:
            cp = es.enter_context(tc.tile_pool(name="const", bufs=1))
            YTp = es.enter_context(tc.tile_pool(name="YTp", bufs=1))
            x2p = es.enter_context(tc.tile_pool(name="x2p", bufs=1))
            mwp = es.enter_context(tc.tile_pool(name="mw", bufs=2))
            wpp = es.enter_context(tc.tile_pool(name="wpp", bufs=1))
            ps1 = es.enter_context(tc.tile_pool(name="ps1", bufs=4, space="PSUM"))
            # PE warm-up: ~5us of junk matmuls so the HAM clock-gate opens
            # before the first real matmul (and stays open through P1's
            # short gaps).  Output is DMA'd out so DCE can't drop it.
            ones_w = cp.tile([P, P], bf16, tag="ones_w")
            nc.vector.memset(ones_w[:], 0.001)
            warm_ps = ps1.tile([P, 128], f32, tag="ps", name="warm_ps")
            for _ in range(48):
                nc.tensor.matmul(out=warm_ps[:], lhsT=ones_w[:], rhs=ones_w[:],
                                 start=True, stop=True)
            warm_sb = cp.tile([1, 1], f32, tag="warm_sb")
            nc.scalar.copy(out=warm_sb[:], in_=warm_ps[0:1, 0:1])
            nc.sync.dma_start(out=warm_d, in_=warm_sb[:])

            # prefetch all heavy weights up-front on otherwise-idle queues:
            # gpsimd queue: MLP strips 0/1 + proj; scalar queue: QK pairs.
            mlp_w = {}
            def load_strip(s2, eng):
                w1t = mwp.tile([P, KC * 1024], bf16, tag="w1")
                eng.dma_start(out=w1t[:],
                              in_=w1_d[:, s2 * (KC * 1024):(s2 + 1) * (KC * 1024)])
                w2t = mwp.tile([P, 8 * C], bf16, tag="w2")
                eng.dma_start(out=w2t[:],
                              in_=w2_d[:, s2 * (8 * C):(s2 + 1) * (8 * C)])
                mlp_w[s2] = (w1t, w2t)
            load_strip(0, nc.gpsimd)
            load_strip(1, nc.gpsimd)
            wpt = wpp.tile([P, KC * C], bf16, tag="wp")
            nc.gpsimd.dma_start(out=wpt[:], in_=wp_d)

            mask01 = cp.tile([P, P], bf16, tag="mask01")
            make_upper_triangular(nc, mask01[:], val=1.0, diag=True)
            epsc = cp.tile([P, 1], f32, tag="epsc")
            nc.vector.memset(epsc[:], LN_EPS)
            ones_c = cp.tile([P, D], bf16, tag="ones_c")
            nc.vector.memset(ones_c[:], 1.0)
            ones_r = cp.tile([1, P], bf16, tag="ones_r")
            nc.vector.memset(ones_r[:], 1.0)

            bqk_t = cp.tile([P, 2 * NP], f32, tag="bqk")
            nc.sync.dma_start(out=bqk_t[:], in_=bqk_d)
            b1c = cp.tile([P, F // P], f32, tag="b1c")
            nc.sync.dma_start(out=b1c[:], in_=b1_d)
            bvb = cp.tile([P, C], f32, tag="bvb")
            bv_bc = bass.AP(tensor=bv_d.tensor, offset=bv_d.offset,
                            ap=[[0, P]] + list(bv_d.ap))
            nc.gpsimd.dma_start(out=bvb[:], in_=bv_bc)
            bp_t = cp.tile([1, C], bf16, tag="bp_t")
            nc.sync.dma_start(out=bp_t[:], in_=bp_d)
            b2_t = cp.tile([1, C], bf16, tag="b2_t")
            nc.sync.dma_start(out=b2_t[:], in_=b2_d)
            if not USE_BIAS_MM:
                bpb = cp.tile([P, C], bf16, tag="bpb")
                nc.gpsimd.dma_start(out=bpb[:], in_=bass.AP(
                    tensor=bp_d.tensor, offset=bp_d.offset,
                    ap=[[0, P], list(bp_d.ap)[1]]))
                b2b = cp.tile([P, C], bf16, tag="b2b")
                nc.gpsimd.dma_start(out=b2b[:], in_=bass.AP(
                    tensor=b2_d.tensor, offset=b2_d.offset,
                    ap=[[0, P], list(b2_d.ap)[1]]))

            YT = [YTp.tile([P, T], bf16, tag=f"YT{k}", name=f"YT{k}") for k in range(KC)]
            x2s = [x2p.tile([P, C], f32, tag=f"x2_{i}", name=f"x2_{i}") for i in range(TT)]

            # ---------------- P1: LN1 + transpose + V ----------------
            att_es = ExitStack()
            hTp = att_es.enter_context(tc.tile_pool(name="hTp", bufs=1))
            vp = att_es.enter_context(tc.tile_pool(name="vp", bufs=1))
            p1_es = ExitStack()
            wvp = p1_es.enter_context(tc.tile_pool(name="wvp", bufs=1))
            p1p = p1_es.enter_context(tc.tile_pool(name="p1", bufs=3))
            p1t = p1_es.enter_context(tc.tile_pool(name="p1t", bufs=4))
            if True:
                hTall = hTp.tile([P, KC, T], bf16, tag="hTall", name="hTall")
                wvt = wvp.tile([P, KC * C], bf16, tag="wv")
                nc.scalar.dma_start(out=wvt[:], in_=wv_d)
                wv3 = wvt[:].rearrange("p (k c) -> p k c", k=KC)
                vts = []
                for i in range(TT):
                    xt = p1p.tile([P, C], f32, tag="xt")
                    nc.sync.dma_start(out=xt[:], in_=x_d[i * P:(i + 1) * P, :])
                    h = p1p.tile([P, C], bf16, tag="h")
                    _layer_norm(nc, p1t, xt[:], h[:], epsc[:])
                    nc.sync.dma_start_transpose(
                        out=hTall[:, :, i * P:(i + 1) * P], in_=h[:])
                    # V for this token tile
                    chA = ps1.tile([P, 512], f32, tag="ps")
                    chB = ps1.tile([P, 256], f32, tag="ps")
                    for k in range(KC):
                        lhsT = hTall[:, k, i * P:(i + 1) * P]
                        nc.tensor.matmul(out=chA[:], lhsT=lhsT, rhs=wv3[:, k, 0:512],
                                         start=(k == 0), stop=(k == KC - 1))
                        nc.tensor.matmul(out=chB[:], lhsT=lhsT, rhs=wv3[:, k, 512:768],
                                         start=(k == 0), stop=(k == KC - 1))
                    vt = vp.tile([P, H * (D + 1)], bf16, tag=f"v{i}", name=f"v{i}")
                    vv = vt[:].rearrange("p (h e) -> p h e", e=D + 1)
                    nc.vector.tensor_add(
                        out=vv[:, 0:8, 0:D],
                        in0=chA[:].rearrange("p (h e) -> p h e", e=D),
                        in1=bvb[:, 0:512].rearrange("p (h e) -> p h e", e=D))
                    nc.vector.tensor_add(
                        out=vv[:, 8:12, 0:D],
                        in0=chB[:].rearrange("p (h e) -> p h e", e=D),
                        in1=bvb[:, 512:768].rearrange("p (h e) -> p h e", e=D))
                    nc.vector.memset(vv[:, :, D:D + 1], 1.0)
                    vts.append(vt)
                p1_es.close()

                # ---------------- P2: attention per head pair ----------------
                waqkp = att_es.enter_context(tc.tile_pool(name="waqk", bufs=2))
                qkp = att_es.enter_context(tc.tile_pool(name="qk", bufs=2))
                attp = att_es.enter_context(tc.tile_pool(name="att", bufs=3))
                rscp = att_es.enter_context(tc.tile_pool(name="rsc", bufs=2))
                ynp = att_es.enter_context(tc.tile_pool(name="yn", bufs=2))
                psyp = att_es.enter_context(tc.tile_pool(name="psy", bufs=2, space="PSUM"))
                if True:
                    for pi in range(NP):
                        wq = waqkp.tile([P, KC * 2 * P], bf16, tag="waqk")
                        nc.scalar.dma_start(
                            out=wq[:],
                            in_=wqk_d[:, pi * (KC * 2 * P):(pi + 1) * (KC * 2 * P)])
                        wq4 = wq[:].rearrange("p (k d m) -> p k d m", d=2, m=P)
                        qT = qkp.tile([P, T], bf16, tag="qT")
                        kT = qkp.tile([P, T], bf16, tag="kT")
                        for dqk, dst in ((0, qT), (1, kT)):
                            for c0 in (0, 512):
                                ch = ps1.tile([P, 512], f32, tag="ps")
                                for k in range(KC):
                                    nc.tensor.matmul(
                                        out=ch[:], lhsT=wq4[:, k, dqk, :],
                                        rhs=hTall[:, k, c0:c0 + 512],
                                        start=(k == 0), stop=(k == KC - 1))
                                nc.vector.tensor_scalar_add(
                                    out=dst[:, c0:c0 + 512], in0=ch[:],
                                    scalar1=bqk_t[:, dqk * NP + pi:dqk * NP + pi + 1])

                        # scores + exp + mask, heads A/B interleaved
                        atts = ([], [])
                        for j in range(TT):
                            nt = (TT - j) * P
                            for hh in range(2):
                                hoff = hh * D
                                at = attp.tile([P, nt], bf16, tag=f"att{j}")
                                for c0 in range(0, nt, 512):
                                    cw = min(512, nt - c0)
                                    ch = ps1.tile([P, 512], f32, tag="ps")
                                    nc.tensor.matmul(
                                        out=ch[:, 0:cw],
                                        lhsT=kT[hoff:hoff + D, j * P:(j + 1) * P],
                                        rhs=qT[hoff:hoff + D, j * P + c0:j * P + c0 + cw],
                                        start=True, stop=True)
                                    nc.scalar.activation(out=at[:, c0:c0 + cw],
                                                         in_=ch[:, 0:cw],
                                                         func=AF.Exp, scale=0.125)
                                eng = nc.gpsimd if USE_GPSIMD_MASK else nc.vector
                                eng.tensor_mul(out=at[:, 0:P], in0=at[:, 0:P],
                                               in1=mask01[:])
                                atts[hh].append(at)

                        for hh in range(2):
                            hg = 2 * pi + hh
                            att_h = atts[hh]
                            yA = psyp.tile([D + 1, 512], f32, tag="yA")
                            yB = psyp.tile([D + 1, 512], f32, tag="yB")
                            for j in range(4):
                                vloc = vts[j][:, hg * (D + 1):(hg + 1) * (D + 1)]
                                nc.tensor.matmul(
                                    out=yA[:, j * P:512], lhsT=vloc,
                                    rhs=att_h[j][:, 0:(4 - j) * P],
                                    start=(j == 0), stop=(j == 3))
                            for j in range(TT):
                                vloc = vts[j][:, hg * (D + 1):(hg + 1) * (D + 1)]
                                c0 = max(j - 4, 0) * P
                                r0 = (max(j, 4) - j) * P
                                nc.tensor.matmul(
                                    out=yB[:, c0:512], lhsT=vloc,
                                    rhs=att_h[j][:, r0:(TT - j) * P],
                                    start=(j == 0), stop=(j == TT - 1))
                            rrow = rscp.tile([D + 1, T], f32, tag="rrow")
                            if USE_RECIP_APPROX:
                                # custom-DVE op mishandles base partition 64;
                                # run at base 0 over all 65 rows (row 64 = the
                                # softmax sums; rows 0-63 are discarded), same
                                # cost -- DVE time scales with the free dim.
                                nc.vector.reciprocal_approx_fast(
                                    out=rrow[:, 0:512], in_=yA[:])
                                nc.vector.reciprocal_approx_fast(
                                    out=rrow[:, 512:1024], in_=yB[:])
                            else:
                                nc.vector.reciprocal(out=rrow[D:D + 1, 0:512],
                                                     in_=yA[D:D + 1, :])
                                nc.vector.reciprocal(out=rrow[D:D + 1, 512:1024],
                                                     in_=yB[D:D + 1, :])
                            rbf = rscp.tile([D + 1, T], bf16, tag="rbf")
                            nc.vector.tensor_copy(out=rbf[D:D + 1, :], in_=rrow[D:D + 1, :])
                            Rsb = rscp.tile([D, T], bf16, tag="Rsb")
                            for c0 in (0, 512):
                                chR = ps1.tile([P, 512], f32, tag="ps")
                                nc.tensor.matmul(out=chR[0:D, :],
                                                 lhsT=ones_c[D:D + 1, :],
                                                 rhs=rbf[D:D + 1, c0:c0 + 512],
                                                 start=True, stop=True)
                                nc.scalar.copy(out=Rsb[:, c0:c0 + 512], in_=chR[0:D, :])
                            if hh == 0:
                                nc.vector.tensor_mul(out=YT[pi][0:D, 0:512],
                                                     in0=yA[0:D, :], in1=Rsb[:, 0:512])
                                nc.vector.tensor_mul(out=YT[pi][0:D, 512:1024],
                                                     in0=yB[0:D, :], in1=Rsb[:, 512:1024])
                            else:
                                ynt = ynp.tile([D, T], bf16, tag="yn")
                                nc.vector.tensor_mul(out=ynt[:, 0:512],
                                                     in0=yA[0:D, :], in1=Rsb[:, 0:512])
                                nc.vector.tensor_mul(out=ynt[:, 512:1024],
                                                     in0=yB[0:D, :], in1=Rsb[:, 512:1024])
                                nc.sync.dma_start(out=YT[pi][D:P, :], in_=ynt[:])

            # ---------------- P3: proj + residual + LN2 ----------------
            att_es.close()
            p34_es = ExitStack()
            h2Tp = p34_es.enter_context(tc.tile_pool(name="h2Tp", bufs=1))
            if True:
                h2all = h2Tp.tile([P, KC, T], bf16, tag="h2all", name="h2all")

                p4p = p34_es.enter_context(tc.tile_pool(name="p4", bufs=3))
                p4t = p34_es.enter_context(tc.tile_pool(name="p4t", bufs=4))
                if True:
                    wp3 = wpt[:].rearrange("p (k c) -> p k c", k=KC)
                    for i in range(TT):
                        xre = p4p.tile([P, C], f32, tag="xre")
                        nc.sync.dma_start(out=xre[:], in_=x_d[i * P:(i + 1) * P, :])
                        chA = ps1.tile([P, 512], f32, tag="ps")
                        chB = ps1.tile([P, 256], f32, tag="ps")
                        for k in range(KC):
                            lhsT = YT[k][:, i * P:(i + 1) * P]
                            lastk = (k == KC - 1) and not USE_BIAS_MM
                            nc.tensor.matmul(out=chA[:], lhsT=lhsT, rhs=wp3[:, k, 0:512],
                                             start=(k == 0), stop=lastk)
                            nc.tensor.matmul(out=chB[:], lhsT=lhsT, rhs=wp3[:, k, 512:768],
                                             start=(k == 0), stop=lastk)
                        if USE_BIAS_MM:
                            nc.tensor.matmul(out=chA[:], lhsT=ones_r[:], rhs=bp_t[:, 0:512],
                                             start=False, stop=True)
                            nc.tensor.matmul(out=chB[:], lhsT=ones_r[:], rhs=bp_t[:, 512:768],
                                             start=False, stop=True)
                        x2 = x2s[i]
                        nc.vector.tensor_add(out=x2[:, 0:512], in0=chA[:], in1=xre[:, 0:512])
                        nc.vector.tensor_add(out=x2[:, 512:768], in0=chB[:], in1=xre[:, 512:768])
                        if not USE_BIAS_MM:
                            nc.vector.tensor_add(out=x2[:], in0=x2[:], in1=bpb[:])
                        h2 = p4p.tile([P, C], bf16, tag="h2")
                        _layer_norm(nc, p4t, x2[:], h2[:], epsc[:])
                        nc.scalar.dma_start_transpose(
                            out=h2all[:, :, i * P:(i + 1) * P], in_=h2[:])

                # ---------------- P4: MLP in 3 strip-pairs ----------------
                gtp = p34_es.enter_context(tc.tile_pool(name="gt", bufs=2))
                accp = p34_es.enter_context(tc.tile_pool(name="accp", bufs=2))
                psfp = p34_es.enter_context(tc.tile_pool(name="psf", bufs=2, space="PSUM"))
                if True:
                    for s2 in range(NS):
                        w1t, w2t = mlp_w[s2]
                        w1v = w1t[:].rearrange("p (k n) -> p k n", k=KC)
                        w2v = w2t[:].rearrange("p (k c) -> p k c", k=8)
                        gts = []
                        for m in range(8):
                            gm = s2 * 8 + m
                            gt = gtp.tile([P, T], bf16, tag=f"gt{m}")
                            for c0 in (0, 512):
                                ch = ps1.tile([P, 512], f32, tag="ps")
                                for k in range(KC):
                                    nc.tensor.matmul(
                                        out=ch[:], lhsT=w1v[:, k, m * P:(m + 1) * P],
                                        rhs=h2all[:, k, c0:c0 + 512],
                                        start=(k == 0), stop=(k == KC - 1))
                                nc.scalar.activation(out=gt[:, c0:c0 + 512], in_=ch[:],
                                                     func=AF.Gelu_apprx_tanh,
                                                     bias=b1c[:, gm:gm + 1], scale=1.0)
                            gts.append(gt)
                        for i in range(TT):
                            psft = psfp.tile([P, C], f32, tag="psf")
                            for kk in range(8):
                                lhsT = gts[kk][:, i * P:(i + 1) * P]
                                last = (kk == 7) and ((s2 != 0) or not USE_BIAS_MM)
                                nc.tensor.matmul(out=psft[:, 0:512], lhsT=lhsT,
                                                 rhs=w2v[:, kk, 0:512],
                                                 start=(kk == 0), stop=last)
                                nc.tensor.matmul(out=psft[:, 512:768], lhsT=lhsT,
                                                 rhs=w2v[:, kk, 512:768],
                                                 start=(kk == 0), stop=last)
                            if s2 == 0 and USE_BIAS_MM:
                                nc.tensor.matmul(out=psft[:, 0:512], lhsT=ones_r[:],
                                                 rhs=b2_t[:, 0:512], start=False, stop=True)
                                nc.tensor.matmul(out=psft[:, 512:768], lhsT=ones_r[:],
                                                 rhs=b2_t[:, 512:768], start=False, stop=True)
                            if s2 == 0 and not USE_BIAS_MM:
                                nc.vector.tensor_add(out=x2s[i][:], in0=x2s[i][:], in1=b2b[:])
                            if s2 < NS - 1:
                                nc.vector.tensor_add(out=x2s[i][:], in0=x2s[i][:], in1=psft[:])
                            else:
                                acc = accp.tile([P, C], f32, tag="acc")
                                nc.vector.tensor_add(out=acc[:], in0=x2s[i][:], in1=psft[:])
                                nc.sync.dma_start(out=out_d[i * P:(i + 1) * P, :], in_=acc[:])
                        if s2 == 0:
                            load_strip(2, nc.gpsimd)
            p34_es.close()

    nc.compile()
    return nc


_NC_CACHE = {}


def _get_nc():
    if "nc" not in _NC_CACHE:
        _NC_CACHE["nc"] = build_nc()
    return _NC_CACHE["nc"]


def _pack_weights(inputs):
    """Fold LN affine params into the adjacent matmuls and repack all
    weights into SBUF-layout [128, ...] host arrays (bf16 for matmul
    operands, fp32 for biases/stats)."""
    def g(n):
        return np.asarray(inputs[n], dtype=np.float32)

    w_attn = g("w_attn") * g("ln1_g")[:, None]
    b_attn = g("b_attn") + g("ln1_b") @ np.asarray(inputs["w_attn"], np.float32)
    w_fc1 = g("w_fc1") * g("ln2_g")[:, None]
    b_fc1 = g("b_fc1") + g("ln2_b") @ np.asarray(inputs["w_fc1"], np.float32)
    w_proj = g("w_proj")
    w_fc2 = g("w_fc2")

    bf = ml_dtypes.bfloat16

    wq_part = w_attn[:, 0:C].reshape(KC, P, NP, P)
    wk_part = w_attn[:, C:2 * C].reshape(KC, P, NP, P)
    wqk = np.stack([wq_part, wk_part], axis=3)          # [k, p, pi, d, m]
    wqk = wqk.transpose(1, 2, 0, 3, 4).reshape(P, NP * KC * 2 * P)

    wv = w_attn[:, 2 * C:].reshape(KC, P, C).transpose(1, 0, 2).reshape(P, KC * C)
    bqk = b_attn[0:2 * C].reshape(2, NP, P).transpose(2, 0, 1).reshape(P, 2 * NP)
    bv = b_attn[2 * C:3 * C]
    wp = w_proj.reshape(KC, P, C).transpose(1, 0, 2).reshape(P, KC * C)
    bp = np.asarray(inputs["b_proj"], np.float32)[None, :]
    w1 = w_fc1.reshape(KC, P, NS, 1024).transpose(1, 2, 0, 3).reshape(P, NS * KC * 1024)
    w2 = w_fc2.reshape(NS, 8, P, C).transpose(2, 0, 1, 3).reshape(P, NS * 8 * C)
    b2 = np.asarray(inputs["b_fc2"], np.float32)[None, :]
    b1p = b_fc1.reshape(F // P, P).T            # b1p[p, m] = b_fc1[m*P + p]

    return {
        "wqk": np.ascontiguousarray(wqk.astype(bf)),
        "wv": np.ascontiguousarray(wv.astype(bf)),
        "bqk": np.ascontiguousarray(bqk),
        "bv": np.ascontiguousarray(bv),
        "wp": np.ascontiguousarray(wp.astype(bf)),
        "bp": np.ascontiguousarray(bp.astype(bf)),
        "w1": np.ascontiguousarray(w1.astype(bf)),
        "b1": np.ascontiguousarray(b1p),
        "w2": np.ascontiguousarray(w2.astype(bf)),
        "b2": np.ascontiguousarray(b2.astype(bf)),
    }


def _make_in_maps(inputs):
    x = np.ascontiguousarray(np.asarray(inputs["x"], dtype=np.float32))
    assert x.shape == (N_CORES, T, C), x.shape
    weights = _pack_weights(inputs)
    in_maps = []
    for c in range(N_CORES):
        m = {"x": np.ascontiguousarray(x[c])}
        m.update(weights)
        in_maps.append(m)
    return in_maps


def kernel(**inputs):
    nc = _get_nc()
    in_maps = _make_in_maps(inputs)
    res = run_bass_kernel_spmd(nc, in_maps, core_ids=list(range(N_CORES)))
    return np.stack([np.asarray(res.results[c]["out"]) for c in range(N_CORES)], axis=0)


if __name__ == "__main__":
    rng = np.random.default_rng(0)
    ins = {
        "x": rng.standard_normal((N_CORES, T, C), dtype=np.float32),
        "ln1_g": np.ones(C, np.float32), "ln1_b": np.zeros(C, np.float32),
        "w_attn": rng.standard_normal((C, 3 * C), dtype=np.float32) * 0.02,
        "b_attn": np.zeros(3 * C, np.float32),
        "w_proj": rng.standard_normal((C, C), dtype=np.float32) * 0.02,
        "b_proj": np.zeros(C, np.float32),
        "ln2_g": np.ones(C, np.float32), "ln2_b": np.zeros(C, np.float32),
        "w_fc1": rng.standard_normal((C, F), dtype=np.float32) * 0.02,
        "b_fc1": np.zeros(F, np.float32),
        "w_fc2": rng.standard_normal((C, F), dtype=np.float32).T.copy() * 0.02,
        "b_fc2": np.zeros(C, np.float32),
    }
    out = kernel(**ins)
    print("out", out.shape, out.dtype, float(np.abs(out).max()))


# revision 28
# speedup vs baseline: 1.2121x; 1.2121x over previous
"""Trainium2 Bass kernel for a GPT-2 style transformer block.

Problem: B=8, T=1024, C=768, H=12 heads, causal attention, GELU-tanh MLP.
Sharding: data-parallel over batch -- one batch element per NeuronCore,
weights replicated, no collectives.

Host-side prep (in kernel(), plain numpy):
  - LN1 gamma/beta folded into w_attn/b_attn; LN2 into w_fc1/b_fc1, so
    on-device LN is just (x - mu) * rsigma (one tensor_scalar op).
  - Weights repacked into SBUF-layout [128, ...] bf16 tensors so each
    weight tile is one large contiguous DMA.

Per-core dataflow (token tiles of 128, feature tiles of 128):
  P1  LN1 (bn_stats/bn_aggr fp32) token-major; DMA-transpose h -> hT
      (feature-major bf16); V = h @ Wv packed per head as [v | ones],
      interleaved per token tile so the PE starts early.
  P2  per head-pair: Q^T,K^T feature-major; scores computed transposed
      S^T[s,t] = K^T.T@Q^T with the two heads of a pair issued to PE
      row-groups 0/64 (K=64 matmuls run concurrently); exp(0.125*S) on
      ACT straight out of PSUM; causal diagonal mask on GpSimd;
      att^T @ [v|ones] gives y^T + softmax row-sums in the same
      matmuls; row-sum reciprocal via DVE reciprocal_approx_fast; PE
      ones-broadcast to all 64 rows; normalize into YT.
  P3  proj token-major, b_proj added via a K=1 ones-matmul, +residual,
      LN2 fused, DMA-transpose h2 -> h2T.
  P4  MLP in 3 strip-pairs (1024 hidden each): fc1+GELU hidden-major,
      fc2 accumulates 8 matmuls in PSUM (b_fc2 via K=1 ones-matmul on
      the first strip-pair), residual accumulated in SBUF fp32.

Matmul operands bf16 (fp32 PSUM); LN stats, residual stream, softmax
reciprocals fp32.
"""

import sys
from contextlib import ExitStack

if "/opt/trn_rl_repo" not in sys.path:
    sys.path.insert(0, "/opt/trn_rl_repo")

import ml_dtypes
import numpy as np

import concourse.bass as bass
import concourse.bacc as bacc
import concourse.mybir as mybir
import concourse.tile as tile
from concourse.bass_utils import run_bass_kernel_spmd
from concourse.bass import _add_dep_helper
from concourse.masks import make_upper_triangular

P = 128
T = 1024
C = 768
H = 12
D = 64
F = 3072
TT = T // P      # 8 token tiles
KC = C // P      # 6 feature tiles
NP = H // 2      # 6 head pairs
NS = 3           # MLP strip-pairs (1024 hidden each)
LN_EPS = 1e-5
f32 = mybir.dt.float32
bf16 = mybir.dt.bfloat16
AF = mybir.ActivationFunctionType
ALU = mybir.AluOpType

N_CORES = 8

INPUT_NAMES = [
    "x", "ln1_g", "ln1_b", "w_attn", "b_attn", "w_proj", "b_proj",
    "ln2_g", "ln2_b", "w_fc1", "b_fc1", "w_fc2", "b_fc2",
]


def _layer_norm(nc, tmp, x_ap, out_h, eps_ap):
    """out_h = (x - mean(x)) * rsqrt(var(x)+eps) over the 768-wide free dim."""
    stats = tmp.tile([P, 3, 6], f32, tag="lnstats")
    xv = x_ap.rearrange("p (a b) -> p a b", b=256)
    for a in range(3):
        nc.vector.bn_stats(out=stats[:, a, :], in_=xv[:, a, :])
    mv = tmp.tile([P, 2], f32, tag="lnmv")
    nc.vector.bn_aggr(out=mv[:], in_=stats[:])
    rs = tmp.tile([P, 1], f32, tag="lnrs")
    nc.scalar.activation(out=rs[:], in_=mv[:, 1:2], func=AF.Sqrt,
                         bias=eps_ap, scale=1.0)
    rsr = tmp.tile([P, 1], f32, tag="lnrsr")
    nc.vector.reciprocal(out=rsr[:], in_=rs[:])
    nc.vector.tensor_scalar(out=out_h, in0=x_ap, scalar1=mv[:, 0:1],
                            scalar2=rsr[:], op0=ALU.subtract, op1=ALU.mult)


USE_GPSIMD_MASK = True
USE_BIAS_MM = True
USE_RECIP_APPROX = True


def build_nc():
    nc = bacc.Bacc("TRN2", target_bir_lowering=False, debug=False)

    x_d = nc.dram_tensor("x", [T, C], f32, kind="ExternalInput").ap()
    wqk_d = nc.dram_tensor("wqk", [P, NP * KC * 2 * P], bf16, kind="ExternalInput").ap()
    wv_d = nc.dram_tensor("wv", [P, KC * C], bf16, kind="ExternalInput").ap()
    bqk_d = nc.dram_tensor("bqk", [P, 2 * NP], f32, kind="ExternalInput").ap()
    bv_d = nc.dram_tensor("bv", [C], f32, kind="ExternalInput").ap()
    wp_d = nc.dram_tensor("wp", [P, KC * C], bf16, kind="ExternalInput").ap()
    bp_d = nc.dram_tensor("bp", [1, C], bf16, kind="ExternalInput").ap()
    w1_d = nc.dram_tensor("w1", [P, NS * KC * 1024], bf16, kind="ExternalInput").ap()
    b1_d = nc.dram_tensor("b1", [P, F // P], f32, kind="ExternalInput").ap()
    w2_d = nc.dram_tensor("w2", [P, NS * 8 * C], bf16, kind="ExternalInput").ap()
    b2_d = nc.dram_tensor("b2", [1, C], bf16, kind="ExternalInput").ap()
    warm_d = nc.dram_tensor("warm", [1, 1], f32, kind="ExternalOutput").ap()
    out_d = nc.dram_tensor("out", [T, C], f32, kind="ExternalOutput").ap()

    with tile.TileContext(nc) as tc, ExitStack() as es:
        if True:
            cp = es.enter_context(tc.tile_pool(name="const", bufs=1))
            YTp = es.enter_context(tc.tile_pool(name="YTp", bufs=1))
            x2p = es.enter_context(tc.tile_pool(name="x2p", bufs=1))
            mwp = es.enter_context(tc.tile_pool(name="mw", bufs=2))
            wpp = es.enter_context(tc.tile_pool(name="wpp", bufs=1))
            ps1 = es.enter_context(tc.tile_pool(name="ps1", bufs=4, space="PSUM"))
            # PE warm-up: ~5us of junk matmuls so the HAM clock-gate opens
            # before the first real matmul (and stays open through P1's
            # short gaps).  Output is DMA'd out so DCE can't drop it.
            ones_w = cp.tile([P, P], bf16, tag="ones_w")
            nc.vector.memset(ones_w[:], 0.001)
            warm_ps = ps1.tile([P, 128], f32, tag="ps", name="warm_ps")
            for _ in range(48):
                nc.tensor.matmul(out=warm_ps[:], lhsT=ones_w[:], rhs=ones_w[:],
                                 start=True, stop=True)
            warm_sb = cp.tile([1, 1], f32, tag="warm_sb")
            nc.scalar.copy(out=warm_sb[:], in_=warm_ps[0:1, 0:1])
            nc.sync.dma_start(out=warm_d, in_=warm_sb[:])

            # V weights early on the gpsimd ring (needed ~15us in); the
            # heavy MLP/proj prefetch also goes on gpsimd but is gated on
            # the last x-tile load so it cannot steal HBM bandwidth from
            # the latency-critical P1 loads.
            wvp = es.enter_context(tc.tile_pool(name="wvp", bufs=1))
            wvt = wvp.tile([P, KC * C], bf16, tag="wv")
            nc.gpsimd.dma_start(out=wvt[:], in_=wv_d)
            mlp_w = {}
            prefetch_insts = []
            def load_strip(s2, eng):
                w1t = mwp.tile([P, KC * 1024], bf16, tag="w1")
                i1 = eng.dma_start(out=w1t[:],
                                   in_=w1_d[:, s2 * (KC * 1024):(s2 + 1) * (KC * 1024)])
                w2t = mwp.tile([P, 8 * C], bf16, tag="w2")
                i2 = eng.dma_start(out=w2t[:],
                                   in_=w2_d[:, s2 * (8 * C):(s2 + 1) * (8 * C)])
                mlp_w[s2] = (w1t, w2t)
                return [i1, i2]
            prefetch_insts += load_strip(0, nc.gpsimd)
            prefetch_insts += load_strip(1, nc.gpsimd)
            wpt = wpp.tile([P, KC * C], bf16, tag="wp")
            prefetch_insts.append(nc.gpsimd.dma_start(out=wpt[:], in_=wp_d))

            mask01 = cp.tile([P, P], bf16, tag="mask01")
            make_upper_triangular(nc, mask01[:], val=1.0, diag=True)
            epsc = cp.tile([P, 1], f32, tag="epsc")
            nc.vector.memset(epsc[:], LN_EPS)
            ones_c = cp.tile([P, D], bf16, tag="ones_c")
            nc.vector.memset(ones_c[:], 1.0)
            ones_r = cp.tile([1, P], bf16, tag="ones_r")
            nc.vector.memset(ones_r[:], 1.0)

            bqk_t = cp.tile([P, 2 * NP], f32, tag="bqk")
            nc.sync.dma_start(out=bqk_t[:], in_=bqk_d)
            b1c = cp.tile([P, F // P], f32, tag="b1c")
            nc.sync.dma_start(out=b1c[:], in_=b1_d)
            bvb = cp.tile([P, C], f32, tag="bvb")
            bv_bc = bass.AP(tensor=bv_d.tensor, offset=bv_d.offset,
                            ap=[[0, P]] + list(bv_d.ap))
            nc.gpsimd.dma_start(out=bvb[:], in_=bv_bc)
            bp_t = cp.tile([1, C], bf16, tag="bp_t")
            nc.sync.dma_start(out=bp_t[:], in_=bp_d)
            b2_t = cp.tile([1, C], bf16, tag="b2_t")
            nc.sync.dma_start(out=b2_t[:], in_=b2_d)
            if not USE_BIAS_MM:
                bpb = cp.tile([P, C], bf16, tag="bpb")
                nc.gpsimd.dma_start(out=bpb[:], in_=bass.AP(
                    tensor=bp_d.tensor, offset=bp_d.offset,
                    ap=[[0, P], list(bp_d.ap)[1]]))
                b2b = cp.tile([P, C], bf16, tag="b2b")
                nc.gpsimd.dma_start(out=b2b[:], in_=bass.AP(
                    tensor=b2_d.tensor, offset=b2_d.offset,
                    ap=[[0, P], list(b2_d.ap)[1]]))

            YT = [YTp.tile([P, T], bf16, tag=f"YT{k}", name=f"YT{k}") for k in range(KC)]
            x2s = [x2p.tile([P, C], f32, tag=f"x2_{i}", name=f"x2_{i}") for i in range(TT)]

            # ---------------- P1: LN1 + transpose + V ----------------
            att_es = ExitStack()
            hTp = att_es.enter_context(tc.tile_pool(name="hTp", bufs=1))
            vp = att_es.enter_context(tc.tile_pool(name="vp", bufs=1))
            p1_es = ExitStack()
            p1p = p1_es.enter_context(tc.tile_pool(name="p1", bufs=3))
            p1t = p1_es.enter_context(tc.tile_pool(name="p1t", bufs=4))
            if True:
                hTall = hTp.tile([P, KC, T], bf16, tag="hTall", name="hTall")
                wv3 = wvt[:].rearrange("p (k c) -> p k c", k=KC)
                vts = []
                xt_last = None
                for i in range(TT):
                    xt = p1p.tile([P, C], f32, tag="xt")
                    xt_last = nc.sync.dma_start(out=xt[:], in_=x_d[i * P:(i + 1) * P, :])
                    h = p1p.tile([P, C], bf16, tag="h")
                    _layer_norm(nc, p1t, xt[:], h[:], epsc[:])
                    nc.sync.dma_start_transpose(
                        out=hTall[:, :, i * P:(i + 1) * P], in_=h[:])
                    # V for this token tile
                    chA = ps1.tile([P, 512], f32, tag="ps")
                    chB = ps1.tile([P, 256], f32, tag="ps")
                    for k in range(KC):
                        lhsT = hTall[:, k, i * P:(i + 1) * P]
                        nc.tensor.matmul(out=chA[:], lhsT=lhsT, rhs=wv3[:, k, 0:512],
                                         start=(k == 0), stop=(k == KC - 1))
                        nc.tensor.matmul(out=chB[:], lhsT=lhsT, rhs=wv3[:, k, 512:768],
                                         start=(k == 0), stop=(k == KC - 1))
                    vt = vp.tile([P, H * (D + 1)], bf16, tag=f"v{i}", name=f"v{i}")
                    vv = vt[:].rearrange("p (h e) -> p h e", e=D + 1)
                    nc.vector.tensor_add(
                        out=vv[:, 0:8, 0:D],
                        in0=chA[:].rearrange("p (h e) -> p h e", e=D),
                        in1=bvb[:, 0:512].rearrange("p (h e) -> p h e", e=D))
                    nc.vector.tensor_add(
                        out=vv[:, 8:12, 0:D],
                        in0=chB[:].rearrange("p (h e) -> p h e", e=D),
                        in1=bvb[:, 512:768].rearrange("p (h e) -> p h e", e=D))
                    nc.vector.memset(vv[:, :, D:D + 1], 1.0)
                    vts.append(vt)
                for pf in prefetch_insts:
                    _add_dep_helper(pf.ins, xt_last.ins, sync=True,
                                    reason="delay MLP prefetch behind P1 x loads")
                p1_es.close()

                # ---------------- P2: attention per head pair ----------------
                waqkp = att_es.enter_context(tc.tile_pool(name="waqk", bufs=2))
                qkp = att_es.enter_context(tc.tile_pool(name="qk", bufs=2))
                attp = att_es.enter_context(tc.tile_pool(name="att", bufs=3))
                rscp = att_es.enter_context(tc.tile_pool(name="rsc", bufs=2))
                ynp = att_es.enter_context(tc.tile_pool(name="yn", bufs=2))
                psyp = att_es.enter_context(tc.tile_pool(name="psy", bufs=2, space="PSUM"))
                if True:
                    for pi in range(NP):
                        wq = waqkp.tile([P, KC * 2 * P], bf16, tag="waqk")
                        nc.scalar.dma_start(
                            out=wq[:],
                            in_=wqk_d[:, pi * (KC * 2 * P):(pi + 1) * (KC * 2 * P)])
                        wq4 = wq[:].rearrange("p (k d m) -> p k d m", d=2, m=P)
                        qT = qkp.tile([P, T], bf16, tag="qT")
                        kT = qkp.tile([P, T], bf16, tag="kT")
                        for dqk, dst in ((0, qT), (1, kT)):
                            for c0 in (0, 512):
                                ch = ps1.tile([P, 512], f32, tag="ps")
                                for k in range(KC):
                                    nc.tensor.matmul(
                                        out=ch[:], lhsT=wq4[:, k, dqk, :],
                                        rhs=hTall[:, k, c0:c0 + 512],
                                        start=(k == 0), stop=(k == KC - 1))
                                nc.vector.tensor_scalar_add(
                                    out=dst[:, c0:c0 + 512], in0=ch[:],
                                    scalar1=bqk_t[:, dqk * NP + pi:dqk * NP + pi + 1])

                        # scores + exp + mask, heads A/B interleaved
                        atts = ([], [])
                        for j in range(TT):
                            nt = (TT - j) * P
                            for hh in range(2):
                                hoff = hh * D
                                at = attp.tile([P, nt], bf16, tag=f"att{j}")
                                for c0 in range(0, nt, 512):
                                    cw = min(512, nt - c0)
                                    ch = ps1.tile([P, 512], f32, tag="ps")
                                    nc.tensor.matmul(
                                        out=ch[:, 0:cw],
                                        lhsT=kT[hoff:hoff + D, j * P:(j + 1) * P],
                                        rhs=qT[hoff:hoff + D, j * P + c0:j * P + c0 + cw],
                                        start=True, stop=True)
                                    nc.scalar.activation(out=at[:, c0:c0 + cw],
                                                         in_=ch[:, 0:cw],
                                                         func=AF.Exp, scale=0.125)
                                eng = nc.gpsimd if USE_GPSIMD_MASK else nc.vector
                                eng.tensor_mul(out=at[:, 0:P], in0=at[:, 0:P],
                                               in1=mask01[:])
                                atts[hh].append(at)

                        for hh in range(2):
                            hg = 2 * pi + hh
                            att_h = atts[hh]
                            yA = psyp.tile([D + 1, 512], f32, tag="yA")
                            yB = psyp.tile([D + 1, 512], f32, tag="yB")
                            for j in range(4):
                                vloc = vts[j][:, hg * (D + 1):(hg + 1) * (D + 1)]
                                nc.tensor.matmul(
                                    out=yA[:, j * P:512], lhsT=vloc,
                                    rhs=att_h[j][:, 0:(4 - j) * P],
                                    start=(j == 0), stop=(j == 3))
                            for j in range(TT):
                                vloc = vts[j][:, hg * (D + 1):(hg + 1) * (D + 1)]
                                c0 = max(j - 4, 0) * P
                                r0 = (max(j, 4) - j) * P
                                nc.tensor.matmul(
                                    out=yB[:, c0:512], lhsT=vloc,
                                    rhs=att_h[j][:, r0:(TT - j) * P],
                                    start=(j == 0), stop=(j == TT - 1))
                            rrow = rscp.tile([D + 1, T], f32, tag="rrow")
                            if USE_RECIP_APPROX:
                                # custom-DVE op mishandles base partition 64;
                                # run at base 0 over all 65 rows (row 64 = the
                                # softmax sums; rows 0-63 are discarded), same
                                # cost -- DVE time scales with the free dim.
                                nc.vector.reciprocal_approx_fast(
                                    out=rrow[:, 0:512], in_=yA[:])
                                nc.vector.reciprocal_approx_fast(
                                    out=rrow[:, 512:1024], in_=yB[:])
                            else:
                                nc.vector.reciprocal(out=rrow[D:D + 1, 0:512],
                                                     in_=yA[D:D + 1, :])
                                nc.vector.reciprocal(out=rrow[D:D + 1, 512:1024],
                                                     in_=yB[D:D + 1, :])
                            rbf = rscp.tile([D + 1, T], bf16, tag="rbf")
                            nc.vector.tensor_copy(out=rbf[D:D + 1, :], in_=rrow[D:D + 1, :])
                            Rsb = rscp.tile([D, T], bf16, tag="Rsb")
                            for c0 in (0, 512):
                                chR = ps1.tile([P, 512], f32, tag="ps")
                                nc.tensor.matmul(out=chR[0:D, :],
                                                 lhsT=ones_c[D:D + 1, :],
                                                 rhs=rbf[D:D + 1, c0:c0 + 512],
                                                 start=True, stop=True)
                                nc.scalar.copy(out=Rsb[:, c0:c0 + 512], in_=chR[0:D, :])
                            if hh == 0:
                                nc.vector.tensor_mul(out=YT[pi][0:D, 0:512],
                                                     in0=yA[0:D, :], in1=Rsb[:, 0:512])
                                nc.vector.tensor_mul(out=YT[pi][0:D, 512:1024],
                                                     in0=yB[0:D, :], in1=Rsb[:, 512:1024])
                            else:
                                ynt = ynp.tile([D, T], bf16, tag="yn")
                                nc.vector.tensor_mul(out=ynt[:, 0:512],
                                                     in0=yA[0:D, :], in1=Rsb[:, 0:512])
                                nc.vector.tensor_mul(out=ynt[:, 512:1024],
                                                     in0=yB[0:D, :], in1=Rsb[:, 512:1024])
                                nc.sync.dma_start(out=YT[pi][D:P, :], in_=ynt[:])

            # ---------------- P3: proj + residual + LN2 ----------------
            att_es.close()
            p34_es = ExitStack()
            h2Tp = p34_es.enter_context(tc.tile_pool(name="h2Tp", bufs=1))
            if True:
                h2all = h2Tp.tile([P, KC, T], bf16, tag="h2all", name="h2all")

                p4p = p34_es.enter_context(tc.tile_pool(name="p4", bufs=3))
                p4t = p34_es.enter_context(tc.tile_pool(name="p4t", bufs=4))
                if True:
                    wp3 = wpt[:].rearrange("p (k c) -> p k c", k=KC)
                    for i in range(TT):
                        xre = p4p.tile([P, C], f32, tag="xre")
                        nc.sync.dma_start(out=xre[:], in_=x_d[i * P:(i + 1) * P, :])
                        chA = ps1.tile([P, 512], f32, tag="ps")
                        chB = ps1.tile([P, 256], f32, tag="ps")
                        for k in range(KC):
                            lhsT = YT[k][:, i * P:(i + 1) * P]
                            lastk = (k == KC - 1) and not USE_BIAS_MM
                            nc.tensor.matmul(out=chA[:], lhsT=lhsT, rhs=wp3[:, k, 0:512],
                                             start=(k == 0), stop=lastk)
                            nc.tensor.matmul(out=chB[:], lhsT=lhsT, rhs=wp3[:, k, 512:768],
                                             start=(k == 0), stop=lastk)
                        if USE_BIAS_MM:
                            nc.tensor.matmul(out=chA[:], lhsT=ones_r[:], rhs=bp_t[:, 0:512],
                                             start=False, stop=True)
                            nc.tensor.matmul(out=chB[:], lhsT=ones_r[:], rhs=bp_t[:, 512:768],
                                             start=False, stop=True)
                        x2 = x2s[i]
                        nc.vector.tensor_add(out=x2[:, 0:512], in0=chA[:], in1=xre[:, 0:512])
                        nc.vector.tensor_add(out=x2[:, 512:768], in0=chB[:], in1=xre[:, 512:768])
                        if not USE_BIAS_MM:
                            nc.vector.tensor_add(out=x2[:], in0=x2[:], in1=bpb[:])
                        h2 = p4p.tile([P, C], bf16, tag="h2")
                        _layer_norm(nc, p4t, x2[:], h2[:], epsc[:])
                        nc.scalar.dma_start_transpose(
                            out=h2all[:, :, i * P:(i + 1) * P], in_=h2[:])

                # ---------------- P4: MLP in 3 strip-pairs ----------------
                gtp = p34_es.enter_context(tc.tile_pool(name="gt", bufs=2))
                accp = p34_es.enter_context(tc.tile_pool(name="accp", bufs=2))
                psfp = p34_es.enter_context(tc.tile_pool(name="psf", bufs=2, space="PSUM"))
                if True:
                    for s2 in range(NS):
                        w1t, w2t = mlp_w[s2]
                        w1v = w1t[:].rearrange("p (k n) -> p k n", k=KC)
                        w2v = w2t[:].rearrange("p (k c) -> p k c", k=8)
                        gts = []
                        for m in range(8):
                            gm = s2 * 8 + m
                            gt = gtp.tile([P, T], bf16, tag=f"gt{m}")
                            for c0 in (0, 512):
                                ch = ps1.tile([P, 512], f32, tag="ps")
                                for k in range(KC):
                                    nc.tensor.matmul(
                                        out=ch[:], lhsT=w1v[:, k, m * P:(m + 1) * P],
                                        rhs=h2all[:, k, c0:c0 + 512],
                                        start=(k == 0), stop=(k == KC - 1))
                                nc.scalar.activation(out=gt[:, c0:c0 + 512], in_=ch[:],
                                                     func=AF.Gelu_apprx_tanh,
                                                     bias=b1c[:, gm:gm + 1], scale=1.0)
                            gts.append(gt)
                        for i in range(TT):
                            psft = psfp.tile([P, C], f32, tag="psf")
                            for kk in range(8):
                                lhsT = gts[kk][:, i * P:(i + 1) * P]
                                last = (kk == 7) and ((s2 != 0) or not USE_BIAS_MM)
                                nc.tensor.matmul(out=psft[:, 0:512], lhsT=lhsT,
                                                 rhs=w2v[:, kk, 0:512],
                                                 start=(kk == 0), stop=last)
                                nc.tensor.matmul(out=psft[:, 512:768], lhsT=lhsT,
                                                 rhs=w2v[:, kk, 512:768],
                                                 start=(kk == 0), stop=last)
                            if s2 == 0 and USE_BIAS_MM:
                                nc.tensor.matmul(out=psft[:, 0:512], lhsT=ones_r[:],
                                                 rhs=b2_t[:, 0:512], start=False, stop=True)
                                nc.tensor.matmul(out=psft[:, 512:768], lhsT=ones_r[:],
                                                 rhs=b2_t[:, 512:768], start=False, stop=True)
                            if s2 == 0 and not USE_BIAS_MM:
                                nc.vector.tensor_add(out=x2s[i][:], in0=x2s[i][:], in1=b2b[:])
                            if s2 < NS - 1:
                                nc.vector.tensor_add(out=x2s[i][:], in0=x2s[i][:], in1=psft[:])
                            else:
                                acc = accp.tile([P, C], f32, tag="acc")
                                nc.vector.tensor_add(out=acc[:], in0=x2s[i][:], in1=psft[:])
                                nc.sync.dma_start(out=out_d[i * P:(i + 1) * P, :], in_=acc[:])
                        if s2 == 0:
                            load_strip(2, nc.gpsimd)
            p34_es.close()

    nc.compile()
    return nc


_NC_CACHE = {}


def _get_nc():
    if "nc" not in _NC_CACHE:
        _NC_CACHE["nc"] = build_nc()
    return _NC_CACHE["nc"]


def _pack_weights(inputs):
    """Fold LN affine params into the adjacent matmuls and repack all
    weights into SBUF-layout [128, ...] host arrays (bf16 for matmul
    operands, fp32 for biases/stats)."""
    def g(n):
        return np.asarray(inputs[n], dtype=np.float32)

    w_attn = g("w_attn") * g("ln1_g")[:, None]
    b_attn = g("b_attn") + g("ln1_b") @ np.asarray(inputs["w_attn"], np.float32)
    w_fc1 = g("w_fc1") * g("ln2_g")[:, None]
    b_fc1 = g("b_fc1") + g("ln2_b") @ np.asarray(inputs["w_fc1"], np.float32)
    w_proj = g("w_proj")
    w_fc2 = g("w_fc2")

    bf = ml_dtypes.bfloat16

    wq_part = w_attn[:, 0:C].reshape(KC, P, NP, P)
    wk_part = w_attn[:, C:2 * C].reshape(KC, P, NP, P)
    wqk = np.stack([wq_part, wk_part], axis=3)          # [k, p, pi, d, m]
    wqk = wqk.transpose(1, 2, 0, 3, 4).reshape(P, NP * KC * 2 * P)

    wv = w_attn[:, 2 * C:].reshape(KC, P, C).transpose(1, 0, 2).reshape(P, KC * C)
    bqk = b_attn[0:2 * C].reshape(2, NP, P).transpose(2, 0, 1).reshape(P, 2 * NP)
    bv = b_attn[2 * C:3 * C]
    wp = w_proj.reshape(KC, P, C).transpose(1, 0, 2).reshape(P, KC * C)
    bp = np.asarray(inputs["b_proj"], np.float32)[None, :]
    w1 = w_fc1.reshape(KC, P, NS, 1024).transpose(1, 2, 0, 3).reshape(P, NS * KC * 1024)
    w2 = w_fc2.reshape(NS, 8, P, C).transpose(2, 0, 1, 3).reshape(P, NS * 8 * C)
    b2 = np.asarray(inputs["b_fc2"], np.float32)[None, :]
    b1p = b_fc1.reshape(F // P, P).T            # b1p[p, m] = b_fc1[m*P + p]

    return {
        "wqk": np.ascontiguousarray(wqk.astype(bf)),
        "wv": np.ascontiguousarray(wv.astype(bf)),
        "bqk": np.ascontiguousarray(bqk),
        "bv": np.ascontiguousarray(bv),
        "wp": np.ascontiguousarray(wp.astype(bf)),
        "bp": np.ascontiguousarray(bp.astype(bf)),
        "w1": np.ascontiguousarray(w1.astype(bf)),
        "b1": np.ascontiguousarray(b1p),
        "w2": np.ascontiguousarray(w2.astype(bf)),
        "b2": np.ascontiguousarray(b2.astype(bf)),
    }


def _make_in_maps(inputs):
    x = np.ascontiguousarray(np.asarray(inputs["x"], dtype=np.float32))
    assert x.shape == (N_CORES, T, C), x.shape
    weights = _pack_weights(inputs)
    in_maps = []
    for c in range(N_CORES):
        m = {"x": np.ascontiguousarray(x[c])}
        m.update(weights)
        in_maps.append(m)
    return in_maps


def kernel(**inputs):
    nc = _get_nc()
    in_maps = _make_in_maps(inputs)
    res = run_bass_kernel_spmd(nc, in_maps, core_ids=list(range(N_CORES)))
    return np.stack([np.asarray(res.results[c]["out"]) for c in range(N_CORES)], axis=0)


if __name__ == "__main__":
    rng = np.random.default_rng(0)
    ins = {
        "x": rng.standard_normal((N_CORES, T, C), dtype=np.float32),
        "ln1_g": np.ones(C, np.float32), "ln1_b": np.zeros(C, np.float32),
        "w_attn": rng.standard_normal((C, 3 * C), dtype=np.float32) * 0.02,
        "b_attn": np.zeros(3 * C, np.float32),
        "w_proj": rng.standard_normal((C, C), dtype=np.float32) * 0.02,
        "b_proj": np.zeros(C, np.float32),
        "ln2_g": np.ones(C, np.float32), "ln2_b": np.zeros(C, np.float32),
        "w_fc1": rng.standard_normal((C, F), dtype=np.float32) * 0.02,
        "b_fc1": np.zeros(F, np.float32),
        "w_fc2": rng.standard_normal((C, F), dtype=np.float32).T.copy() * 0.02,
        "b_fc2": np.zeros(C, np.float32),
    }
    out = kernel(**ins)
    print("out", out.shape, out.dtype, float(np.abs(out).max()))


# revision 29
# speedup vs baseline: 1.2528x; 1.0336x over previous
"""Trainium2 Bass kernel for a GPT-2 style transformer block.

Problem: B=8, T=1024, C=768, H=12 heads, causal attention, GELU-tanh MLP.
Sharding: data-parallel over batch -- one batch element per NeuronCore,
weights replicated, no collectives.

Host-side prep (in kernel(), plain numpy):
  - LN1 gamma/beta folded into w_attn/b_attn; LN2 into w_fc1/b_fc1, so
    on-device LN is just (x - mu) * rsigma (one tensor_scalar op).
  - Weights repacked into SBUF-layout [128, ...] bf16 tensors so each
    weight tile is one large contiguous DMA.

Per-core dataflow (token tiles of 128, feature tiles of 128):
  P1  LN1 (bn_stats/bn_aggr fp32) token-major; DMA-transpose h -> hT
      (feature-major bf16); V = h @ Wv packed per head as [v | ones],
      interleaved per token tile so the PE starts early.
  P2  per head-pair: Q^T,K^T feature-major; scores computed transposed
      S^T[s,t] = K^T.T@Q^T with the two heads of a pair issued to PE
      row-groups 0/64 (K=64 matmuls run concurrently); exp(0.125*S) on
      ACT straight out of PSUM; causal diagonal mask on GpSimd;
      att^T @ [v|ones] gives y^T + softmax row-sums in the same
      matmuls; row-sum reciprocal via DVE reciprocal_approx_fast; PE
      ones-broadcast to all 64 rows; normalize into YT.
  P3  proj token-major, b_proj added via a K=1 ones-matmul, +residual,
      LN2 fused, DMA-transpose h2 -> h2T.
  P4  MLP in 3 strip-pairs (1024 hidden each): fc1+GELU hidden-major,
      fc2 accumulates 8 matmuls in PSUM (b_fc2 via K=1 ones-matmul on
      the first strip-pair), residual accumulated in SBUF fp32.

Matmul operands bf16 (fp32 PSUM); LN stats, residual stream, softmax
reciprocals fp32.
"""

import sys
from contextlib import ExitStack

if "/opt/trn_rl_repo" not in sys.path:
    sys.path.insert(0, "/opt/trn_rl_repo")

import ml_dtypes
import numpy as np

import concourse.bass as bass
import concourse.bacc as bacc
import concourse.mybir as mybir
import concourse.tile as tile
from concourse.bass_utils import run_bass_kernel_spmd
from concourse.bass import _add_dep_helper
from concourse.masks import make_upper_triangular

P = 128
T = 1024
C = 768
H = 12
D = 64
F = 3072
TT = T // P      # 8 token tiles
KC = C // P      # 6 feature tiles
NP = H // 2      # 6 head pairs
NS = 3           # MLP strip-pairs (1024 hidden each)
LN_EPS = 1e-5
f32 = mybir.dt.float32
bf16 = mybir.dt.bfloat16
AF = mybir.ActivationFunctionType
ALU = mybir.AluOpType

N_CORES = 8

INPUT_NAMES = [
    "x", "ln1_g", "ln1_b", "w_attn", "b_attn", "w_proj", "b_proj",
    "ln2_g", "ln2_b", "w_fc1", "b_fc1", "w_fc2", "b_fc2",
]


def _layer_norm(nc, tmp, x_ap, out_h, eps_ap):
    """out_h = (x - mean(x)) * rsqrt(var(x)+eps) over the 768-wide free dim."""
    stats = tmp.tile([P, 3, 6], f32, tag="lnstats")
    xv = x_ap.rearrange("p (a b) -> p a b", b=256)
    for a in range(3):
        nc.vector.bn_stats(out=stats[:, a, :], in_=xv[:, a, :])
    mv = tmp.tile([P, 2], f32, tag="lnmv")
    nc.vector.bn_aggr(out=mv[:], in_=stats[:])
    rs = tmp.tile([P, 1], f32, tag="lnrs")
    nc.scalar.activation(out=rs[:], in_=mv[:, 1:2], func=AF.Sqrt,
                         bias=eps_ap, scale=1.0)
    rsr = tmp.tile([P, 1], f32, tag="lnrsr")
    nc.vector.reciprocal(out=rsr[:], in_=rs[:])
    nc.vector.tensor_scalar(out=out_h, in0=x_ap, scalar1=mv[:, 0:1],
                            scalar2=rsr[:], op0=ALU.subtract, op1=ALU.mult)


USE_GPSIMD_MASK = True
USE_BIAS_MM = True
USE_RECIP_APPROX = True


def build_nc():
    nc = bacc.Bacc("TRN2", target_bir_lowering=False, debug=False)

    x_d = nc.dram_tensor("x", [T, C], f32, kind="ExternalInput").ap()
    wqk_d = nc.dram_tensor("wqk", [P, NP * KC * 2 * P], bf16, kind="ExternalInput").ap()
    wv_d = nc.dram_tensor("wv", [P, KC * C], bf16, kind="ExternalInput").ap()
    bqk_d = nc.dram_tensor("bqk", [P, 2 * NP], f32, kind="ExternalInput").ap()
    bv_d = nc.dram_tensor("bv", [C], f32, kind="ExternalInput").ap()
    wp_d = nc.dram_tensor("wp", [P, KC * C], bf16, kind="ExternalInput").ap()
    bp_d = nc.dram_tensor("bp", [1, C], bf16, kind="ExternalInput").ap()
    w1_d = nc.dram_tensor("w1", [P, NS * KC * 1024], bf16, kind="ExternalInput").ap()
    b1_d = nc.dram_tensor("b1", [P, F // P], f32, kind="ExternalInput").ap()
    w2_d = nc.dram_tensor("w2", [P, NS * 8 * C], bf16, kind="ExternalInput").ap()
    b2_d = nc.dram_tensor("b2", [1, C], bf16, kind="ExternalInput").ap()
    warm_d = nc.dram_tensor("warm", [1, 1], f32, kind="ExternalOutput").ap()
    out_d = nc.dram_tensor("out", [T, C], f32, kind="ExternalOutput").ap()

    with tile.TileContext(nc) as tc, ExitStack() as es:
        if True:
            cp = es.enter_context(tc.tile_pool(name="const", bufs=1))
            YTp = es.enter_context(tc.tile_pool(name="YTp", bufs=1))
            x2p = es.enter_context(tc.tile_pool(name="x2p", bufs=1))
            mwp = es.enter_context(tc.tile_pool(name="mw", bufs=2))
            wpp = es.enter_context(tc.tile_pool(name="wpp", bufs=1))
            ps1 = es.enter_context(tc.tile_pool(name="ps1", bufs=4, space="PSUM"))
            # PE warm-up: ~5us of junk matmuls so the HAM clock-gate opens
            # before the first real matmul (and stays open through P1's
            # short gaps).  Output is DMA'd out so DCE can't drop it.
            ones_w = cp.tile([P, P], bf16, tag="ones_w")
            nc.vector.memset(ones_w[:], 0.001)
            warm_ps = ps1.tile([P, 128], f32, tag="ps", name="warm_ps")
            for _ in range(64):
                nc.tensor.matmul(out=warm_ps[:], lhsT=ones_w[:], rhs=ones_w[:],
                                 start=True, stop=True)
            warm_sb = cp.tile([1, 1], f32, tag="warm_sb")
            nc.scalar.copy(out=warm_sb[:], in_=warm_ps[0:1, 0:1])
            nc.sync.dma_start(out=warm_d, in_=warm_sb[:])

            # V weights early on the gpsimd ring (needed ~15us in); the
            # heavy MLP/proj prefetch also goes on gpsimd but is gated on
            # the last x-tile load so it cannot steal HBM bandwidth from
            # the latency-critical P1 loads.
            wvp = es.enter_context(tc.tile_pool(name="wvp", bufs=1))
            wvt = wvp.tile([P, KC * C], bf16, tag="wv")
            half = KC * C // 2
            nc.gpsimd.dma_start(out=wvt[:, 0:half], in_=wv_d[:, 0:half])
            nc.scalar.dma_start(out=wvt[:, half:], in_=wv_d[:, half:])
            mlp_w = {}
            prefetch_insts = []
            def load_strip(s2, eng):
                w1t = mwp.tile([P, KC * 1024], bf16, tag="w1")
                i1 = eng.dma_start(out=w1t[:],
                                   in_=w1_d[:, s2 * (KC * 1024):(s2 + 1) * (KC * 1024)])
                w2t = mwp.tile([P, 8 * C], bf16, tag="w2")
                i2 = eng.dma_start(out=w2t[:],
                                   in_=w2_d[:, s2 * (8 * C):(s2 + 1) * (8 * C)])
                mlp_w[s2] = (w1t, w2t)
                return [i1, i2]
            prefetch_insts += load_strip(0, nc.gpsimd)
            prefetch_insts += load_strip(1, nc.gpsimd)
            wpt = wpp.tile([P, KC * C], bf16, tag="wp")
            prefetch_insts.append(nc.gpsimd.dma_start(out=wpt[:], in_=wp_d))

            mask01 = cp.tile([P, P], bf16, tag="mask01")
            make_upper_triangular(nc, mask01[:], val=1.0, diag=True)
            epsc = cp.tile([P, 1], f32, tag="epsc")
            nc.vector.memset(epsc[:], LN_EPS)
            ones_c = cp.tile([P, D], bf16, tag="ones_c")
            nc.vector.memset(ones_c[:], 1.0)
            ones_r = cp.tile([1, P], bf16, tag="ones_r")
            nc.vector.memset(ones_r[:], 1.0)

            bqk_t = cp.tile([P, 2 * NP], f32, tag="bqk")
            nc.sync.dma_start(out=bqk_t[:], in_=bqk_d)
            b1c = cp.tile([P, F // P], f32, tag="b1c")
            nc.sync.dma_start(out=b1c[:], in_=b1_d)
            bvb = cp.tile([P, C], f32, tag="bvb")
            bv_bc = bass.AP(tensor=bv_d.tensor, offset=bv_d.offset,
                            ap=[[0, P]] + list(bv_d.ap))
            nc.scalar.dma_start(out=bvb[:], in_=bv_bc)
            bp_t = cp.tile([1, C], bf16, tag="bp_t")
            nc.sync.dma_start(out=bp_t[:], in_=bp_d)
            b2_t = cp.tile([1, C], bf16, tag="b2_t")
            nc.sync.dma_start(out=b2_t[:], in_=b2_d)
            if not USE_BIAS_MM:
                bpb = cp.tile([P, C], bf16, tag="bpb")
                nc.gpsimd.dma_start(out=bpb[:], in_=bass.AP(
                    tensor=bp_d.tensor, offset=bp_d.offset,
                    ap=[[0, P], list(bp_d.ap)[1]]))
                b2b = cp.tile([P, C], bf16, tag="b2b")
                nc.gpsimd.dma_start(out=b2b[:], in_=bass.AP(
                    tensor=b2_d.tensor, offset=b2_d.offset,
                    ap=[[0, P], list(b2_d.ap)[1]]))

            YT = [YTp.tile([P, T], bf16, tag=f"YT{k}", name=f"YT{k}") for k in range(KC)]
            x2s = [x2p.tile([P, C], f32, tag=f"x2_{i}", name=f"x2_{i}") for i in range(TT)]

            # ---------------- P1: LN1 + transpose + V ----------------
            att_es = ExitStack()
            hTp = att_es.enter_context(tc.tile_pool(name="hTp", bufs=1))
            vp = att_es.enter_context(tc.tile_pool(name="vp", bufs=1))
            p1_es = ExitStack()
            p1p = p1_es.enter_context(tc.tile_pool(name="p1", bufs=3))
            p1t = p1_es.enter_context(tc.tile_pool(name="p1t", bufs=4))
            if True:
                hTall = hTp.tile([P, KC, T], bf16, tag="hTall", name="hTall")
                wv3 = wvt[:].rearrange("p (k c) -> p k c", k=KC)
                vts = []
                xt_last = None
                for i in range(TT):
                    xt = p1p.tile([P, C], f32, tag="xt")
                    xt_last = nc.sync.dma_start(out=xt[:], in_=x_d[i * P:(i + 1) * P, :])
                    h = p1p.tile([P, C], bf16, tag="h")
                    _layer_norm(nc, p1t, xt[:], h[:], epsc[:])
                    nc.sync.dma_start_transpose(
                        out=hTall[:, :, i * P:(i + 1) * P], in_=h[:])
                    # V for this token tile
                    chA = ps1.tile([P, 512], f32, tag="ps")
                    chB = ps1.tile([P, 256], f32, tag="ps")
                    for k in range(KC):
                        lhsT = hTall[:, k, i * P:(i + 1) * P]
                        nc.tensor.matmul(out=chA[:], lhsT=lhsT, rhs=wv3[:, k, 0:512],
                                         start=(k == 0), stop=(k == KC - 1))
                        nc.tensor.matmul(out=chB[:], lhsT=lhsT, rhs=wv3[:, k, 512:768],
                                         start=(k == 0), stop=(k == KC - 1))
                    vt = vp.tile([P, H * (D + 1)], bf16, tag=f"v{i}", name=f"v{i}")
                    vv = vt[:].rearrange("p (h e) -> p h e", e=D + 1)
                    nc.vector.tensor_add(
                        out=vv[:, 0:8, 0:D],
                        in0=chA[:].rearrange("p (h e) -> p h e", e=D),
                        in1=bvb[:, 0:512].rearrange("p (h e) -> p h e", e=D))
                    nc.vector.tensor_add(
                        out=vv[:, 8:12, 0:D],
                        in0=chB[:].rearrange("p (h e) -> p h e", e=D),
                        in1=bvb[:, 512:768].rearrange("p (h e) -> p h e", e=D))
                    nc.vector.memset(vv[:, :, D:D + 1], 1.0)
                    vts.append(vt)
                for pf in prefetch_insts:
                    _add_dep_helper(pf.ins, xt_last.ins, sync=True,
                                    reason="delay MLP prefetch behind P1 x loads")
                p1_es.close()

                # ---------------- P2: attention per head pair ----------------
                waqkp = att_es.enter_context(tc.tile_pool(name="waqk", bufs=3))
                qkp = att_es.enter_context(tc.tile_pool(name="qk", bufs=2))
                attp = att_es.enter_context(tc.tile_pool(name="att", bufs=3))
                rscp = att_es.enter_context(tc.tile_pool(name="rsc", bufs=2))
                ynp = att_es.enter_context(tc.tile_pool(name="yn", bufs=2))
                psyp = att_es.enter_context(tc.tile_pool(name="psy", bufs=2, space="PSUM"))
                if True:
                    for pi in range(NP):
                        wq = waqkp.tile([P, KC * 2 * P], bf16, tag="waqk")
                        nc.scalar.dma_start(
                            out=wq[:],
                            in_=wqk_d[:, pi * (KC * 2 * P):(pi + 1) * (KC * 2 * P)])
                        wq4 = wq[:].rearrange("p (k d m) -> p k d m", d=2, m=P)
                        qT = qkp.tile([P, T], bf16, tag="qT")
                        kT = qkp.tile([P, T], bf16, tag="kT")
                        for dqk, dst in ((0, qT), (1, kT)):
                            for c0 in (0, 512):
                                ch = ps1.tile([P, 512], f32, tag="ps")
                                for k in range(KC):
                                    nc.tensor.matmul(
                                        out=ch[:], lhsT=wq4[:, k, dqk, :],
                                        rhs=hTall[:, k, c0:c0 + 512],
                                        start=(k == 0), stop=(k == KC - 1))
                                nc.vector.tensor_scalar_add(
                                    out=dst[:, c0:c0 + 512], in0=ch[:],
                                    scalar1=bqk_t[:, dqk * NP + pi:dqk * NP + pi + 1])

                        # scores + exp + mask, heads A/B interleaved
                        atts = ([], [])
                        for j in range(TT):
                            nt = (TT - j) * P
                            for hh in range(2):
                                hoff = hh * D
                                at = attp.tile([P, nt], bf16, tag=f"att{j}")
                                for c0 in range(0, nt, 512):
                                    cw = min(512, nt - c0)
                                    ch = ps1.tile([P, 512], f32, tag="ps")
                                    nc.tensor.matmul(
                                        out=ch[:, 0:cw],
                                        lhsT=kT[hoff:hoff + D, j * P:(j + 1) * P],
                                        rhs=qT[hoff:hoff + D, j * P + c0:j * P + c0 + cw],
                                        start=True, stop=True)
                                    nc.scalar.activation(out=at[:, c0:c0 + cw],
                                                         in_=ch[:, 0:cw],
                                                         func=AF.Exp, scale=0.125)
                                eng = nc.gpsimd if USE_GPSIMD_MASK else nc.vector
                                eng.tensor_mul(out=at[:, 0:P], in0=at[:, 0:P],
                                               in1=mask01[:])
                                atts[hh].append(at)

                        for hh in range(2):
                            hg = 2 * pi + hh
                            att_h = atts[hh]
                            yA = psyp.tile([D + 1, 512], f32, tag="yA")
                            yB = psyp.tile([D + 1, 512], f32, tag="yB")
                            for j in range(4):
                                vloc = vts[j][:, hg * (D + 1):(hg + 1) * (D + 1)]
                                nc.tensor.matmul(
                                    out=yA[:, j * P:512], lhsT=vloc,
                                    rhs=att_h[j][:, 0:(4 - j) * P],
                                    start=(j == 0), stop=(j == 3))
                            for j in range(TT):
                                vloc = vts[j][:, hg * (D + 1):(hg + 1) * (D + 1)]
                                c0 = max(j - 4, 0) * P
                                r0 = (max(j, 4) - j) * P
                                nc.tensor.matmul(
                                    out=yB[:, c0:512], lhsT=vloc,
                                    rhs=att_h[j][:, r0:(TT - j) * P],
                                    start=(j == 0), stop=(j == TT - 1))
                            rrow = rscp.tile([D + 1, T], f32, tag="rrow")
                            if USE_RECIP_APPROX:
                                # custom-DVE op mishandles base partition 64;
                                # run at base 0 over all 65 rows (row 64 = the
                                # softmax sums; rows 0-63 are discarded), same
                                # cost -- DVE time scales with the free dim.
                                nc.vector.reciprocal_approx_fast(
                                    out=rrow[:, 0:512], in_=yA[:])
                                nc.vector.reciprocal_approx_fast(
                                    out=rrow[:, 512:1024], in_=yB[:])
                            else:
                                nc.vector.reciprocal(out=rrow[D:D + 1, 0:512],
                                                     in_=yA[D:D + 1, :])
                                nc.vector.reciprocal(out=rrow[D:D + 1, 512:1024],
                                                     in_=yB[D:D + 1, :])
                            rbf = rscp.tile([D + 1, T], bf16, tag="rbf")
                            nc.vector.tensor_copy(out=rbf[D:D + 1, :], in_=rrow[D:D + 1, :])
                            Rsb = rscp.tile([D, T], bf16, tag="Rsb")
                            for c0 in (0, 512):
                                chR = ps1.tile([P, 512], f32, tag="ps")
                                nc.tensor.matmul(out=chR[0:D, :],
                                                 lhsT=ones_c[D:D + 1, :],
                                                 rhs=rbf[D:D + 1, c0:c0 + 512],
                                                 start=True, stop=True)
                                nc.scalar.copy(out=Rsb[:, c0:c0 + 512], in_=chR[0:D, :])
                            if hh == 0:
                                nc.vector.tensor_mul(out=YT[pi][0:D, 0:512],
                                                     in0=yA[0:D, :], in1=Rsb[:, 0:512])
                                nc.vector.tensor_mul(out=YT[pi][0:D, 512:1024],
                                                     in0=yB[0:D, :], in1=Rsb[:, 512:1024])
                            else:
                                ynt = ynp.tile([D, T], bf16, tag="yn")
                                nc.vector.tensor_mul(out=ynt[:, 0:512],
                                                     in0=yA[0:D, :], in1=Rsb[:, 0:512])
                                nc.vector.tensor_mul(out=ynt[:, 512:1024],
                                                     in0=yB[0:D, :], in1=Rsb[:, 512:1024])
                                nc.sync.dma_start(out=YT[pi][D:P, :], in_=ynt[:])

            # ---------------- P3: proj + residual + LN2 ----------------
            att_es.close()
            p34_es = ExitStack()
            h2Tp = p34_es.enter_context(tc.tile_pool(name="h2Tp", bufs=1))
            if True:
                h2all = h2Tp.tile([P, KC, T], bf16, tag="h2all", name="h2all")

                p4p = p34_es.enter_context(tc.tile_pool(name="p4", bufs=3))
                p4t = p34_es.enter_context(tc.tile_pool(name="p4t", bufs=4))
                if True:
                    wp3 = wpt[:].rearrange("p (k c) -> p k c", k=KC)
                    for i in range(TT):
                        xre = p4p.tile([P, C], f32, tag="xre")
                        nc.sync.dma_start(out=xre[:], in_=x_d[i * P:(i + 1) * P, :])
                        chA = ps1.tile([P, 512], f32, tag="ps")
                        chB = ps1.tile([P, 256], f32, tag="ps")
                        for k in range(KC):
                            lhsT = YT[k][:, i * P:(i + 1) * P]
                            lastk = (k == KC - 1) and not USE_BIAS_MM
                            nc.tensor.matmul(out=chA[:], lhsT=lhsT, rhs=wp3[:, k, 0:512],
                                             start=(k == 0), stop=lastk)
                            nc.tensor.matmul(out=chB[:], lhsT=lhsT, rhs=wp3[:, k, 512:768],
                                             start=(k == 0), stop=lastk)
                        if USE_BIAS_MM:
                            nc.tensor.matmul(out=chA[:], lhsT=ones_r[:], rhs=bp_t[:, 0:512],
                                             start=False, stop=True)
                            nc.tensor.matmul(out=chB[:], lhsT=ones_r[:], rhs=bp_t[:, 512:768],
                                             start=False, stop=True)
                        x2 = x2s[i]
                        nc.vector.tensor_add(out=x2[:, 0:512], in0=chA[:], in1=xre[:, 0:512])
                        nc.vector.tensor_add(out=x2[:, 512:768], in0=chB[:], in1=xre[:, 512:768])
                        if not USE_BIAS_MM:
                            nc.vector.tensor_add(out=x2[:], in0=x2[:], in1=bpb[:])
                        h2 = p4p.tile([P, C], bf16, tag="h2")
                        _layer_norm(nc, p4t, x2[:], h2[:], epsc[:])
                        nc.scalar.dma_start_transpose(
                            out=h2all[:, :, i * P:(i + 1) * P], in_=h2[:])

                # ---------------- P4: MLP in 3 strip-pairs ----------------
                gtp = p34_es.enter_context(tc.tile_pool(name="gt", bufs=2))
                accp = p34_es.enter_context(tc.tile_pool(name="accp", bufs=2))
                psfp = p34_es.enter_context(tc.tile_pool(name="psf", bufs=2, space="PSUM"))
                if True:
                    for s2 in range(NS):
                        w1t, w2t = mlp_w[s2]
                        w1v = w1t[:].rearrange("p (k n) -> p k n", k=KC)
                        w2v = w2t[:].rearrange("p (k c) -> p k c", k=8)
                        gts = []
                        for m in range(8):
                            gm = s2 * 8 + m
                            gt = gtp.tile([P, T], bf16, tag=f"gt{m}")
                            for c0 in (0, 512):
                                ch = ps1.tile([P, 512], f32, tag="ps")
                                for k in range(KC):
                                    nc.tensor.matmul(
                                        out=ch[:], lhsT=w1v[:, k, m * P:(m + 1) * P],
                                        rhs=h2all[:, k, c0:c0 + 512],
                                        start=(k == 0), stop=(k == KC - 1))
                                nc.scalar.activation(out=gt[:, c0:c0 + 512], in_=ch[:],
                                                     func=AF.Gelu_apprx_tanh,
                                                     bias=b1c[:, gm:gm + 1], scale=1.0)
                            gts.append(gt)
                        for i in range(TT):
                            psft = psfp.tile([P, C], f32, tag="psf")
                            for kk in range(8):
                                lhsT = gts[kk][:, i * P:(i + 1) * P]
                                last = (kk == 7) and ((s2 != 0) or not USE_BIAS_MM)
                                nc.tensor.matmul(out=psft[:, 0:512], lhsT=lhsT,
                                                 rhs=w2v[:, kk, 0:512],
                                                 start=(kk == 0), stop=last)
                                nc.tensor.matmul(out=psft[:, 512:768], lhsT=lhsT,
                                                 rhs=w2v[:, kk, 512:768],
                                                 start=(kk == 0), stop=last)
                            if s2 == 0 and USE_BIAS_MM:
                                nc.tensor.matmul(out=psft[:, 0:512], lhsT=ones_r[:],
                                                 rhs=b2_t[:, 0:512], start=False, stop=True)
                                nc.tensor.matmul(out=psft[:, 512:768], lhsT=ones_r[:],
                                                 rhs=b2_t[:, 512:768], start=False, stop=True)
                            if s2 == 0 and not USE_BIAS_MM:
                                nc.vector.tensor_add(out=x2s[i][:], in0=x2s[i][:], in1=b2b[:])
                            if s2 < NS - 1:
                                nc.vector.tensor_add(out=x2s[i][:], in0=x2s[i][:], in1=psft[:])
                            else:
                                acc = accp.tile([P, C], f32, tag="acc")
                                nc.vector.tensor_add(out=acc[:], in0=x2s[i][:], in1=psft[:])
                                nc.sync.dma_start(out=out_d[i * P:(i + 1) * P, :], in_=acc[:])
                        if s2 == 0:
                            load_strip(2, nc.gpsimd)
            p34_es.close()

    nc.compile()
    return nc


_NC_CACHE = {}


def _get_nc():
    if "nc" not in _NC_CACHE:
        _NC_CACHE["nc"] = build_nc()
    return _NC_CACHE["nc"]


def _pack_weights(inputs):
    """Fold LN affine params into the adjacent matmuls and repack all
    weights into SBUF-layout [128, ...] host arrays (bf16 for matmul
    operands, fp32 for biases/stats)."""
    def g(n):
        return np.asarray(inputs[n], dtype=np.float32)

    w_attn = g("w_attn") * g("ln1_g")[:, None]
    b_attn = g("b_attn") + g("ln1_b") @ np.asarray(inputs["w_attn"], np.float32)
    w_fc1 = g("w_fc1") * g("ln2_g")[:, None]
    b_fc1 = g("b_fc1") + g("ln2_b") @ np.asarray(inputs["w_fc1"], np.float32)
    w_proj = g("w_proj")
    w_fc2 = g("w_fc2")

    bf = ml_dtypes.bfloat16

    wq_part = w_attn[:, 0:C].reshape(KC, P, NP, P)
    wk_part = w_attn[:, C:2 * C].reshape(KC, P, NP, P)
    wqk = np.stack([wq_part, wk_part], axis=3)          # [k, p, pi, d, m]
    wqk = wqk.transpose(1, 2, 0, 3, 4).reshape(P, NP * KC * 2 * P)

    wv = w_attn[:, 2 * C:].reshape(KC, P, C).transpose(1, 0, 2).reshape(P, KC * C)
    bqk = b_attn[0:2 * C].reshape(2, NP, P).transpose(2, 0, 1).reshape(P, 2 * NP)
    bv = b_attn[2 * C:3 * C]
    wp = w_proj.reshape(KC, P, C).transpose(1, 0, 2).reshape(P, KC * C)
    bp = np.asarray(inputs["b_proj"], np.float32)[None, :]
    w1 = w_fc1.reshape(KC, P, NS, 1024).transpose(1, 2, 0, 3).reshape(P, NS * KC * 1024)
    w2 = w_fc2.reshape(NS, 8, P, C).transpose(2, 0, 1, 3).reshape(P, NS * 8 * C)
    b2 = np.asarray(inputs["b_fc2"], np.float32)[None, :]
    b1p = b_fc1.reshape(F // P, P).T            # b1p[p, m] = b_fc1[m*P + p]

    return {
        "wqk": np.ascontiguousarray(wqk.astype(bf)),
        "wv": np.ascontiguousarray(wv.astype(bf)),
        "bqk": np.ascontiguousarray(bqk),
        "bv": np.ascontiguousarray(bv),
        "wp": np.ascontiguousarray(wp.astype(bf)),
        "bp": np.ascontiguousarray(bp.astype(bf)),
        "w1": np.ascontiguousarray(w1.astype(bf)),
        "b1": np.ascontiguousarray(b1p),
        "w2": np.ascontiguousarray(w2.astype(bf)),
        "b2": np.ascontiguousarray(b2.astype(bf)),
    }


def _make_in_maps(inputs):
    x = np.ascontiguousarray(np.asarray(inputs["x"], dtype=np.float32))
    assert x.shape == (N_CORES, T, C), x.shape
    weights = _pack_weights(inputs)
    in_maps = []
    for c in range(N_CORES):
        m = {"x": np.ascontiguousarray(x[c])}
        m.update(weights)
        in_maps.append(m)
    return in_maps


def kernel(**inputs):
    nc = _get_nc()
    in_maps = _make_in_maps(inputs)
    res = run_bass_kernel_spmd(nc, in_maps, core_ids=list(range(N_CORES)))
    return np.stack([np.asarray(res.results[c]["out"]) for c in range(N_CORES)], axis=0)


if __name__ == "__main__":
    rng = np.random.default_rng(0)
    ins = {
        "x": rng.standard_normal((N_CORES, T, C), dtype=np.float32),
        "ln1_g": np.ones(C, np.float32), "ln1_b": np.zeros(C, np.float32),
        "w_attn": rng.standard_normal((C, 3 * C), dtype=np.float32) * 0.02,
        "b_attn": np.zeros(3 * C, np.float32),
        "w_proj": rng.standard_normal((C, C), dtype=np.float32) * 0.02,
        "b_proj": np.zeros(C, np.float32),
        "ln2_g": np.ones(C, np.float32), "ln2_b": np.zeros(C, np.float32),
        "w_fc1": rng.standard_normal((C, F), dtype=np.float32) * 0.02,
        "b_fc1": np.zeros(F, np.float32),
        "w_fc2": rng.standard_normal((C, F), dtype=np.float32).T.copy() * 0.02,
        "b_fc2": np.zeros(C, np.float32),
    }
    out = kernel(**ins)
    print("out", out.shape, out.dtype, float(np.abs(out).max()))
